# revision 8
# baseline (speedup 1.0000x reference)
"""Trainium2 Bass kernel: 2-layer GCN (PyG GCNConv w/ edge weights) + MLP head.

Sharding: core k owns destination nodes [k*NPC, (k+1)*NPC) — which is exactly
graph k (NODES_PER_G == N/8).  All edges are bucketed by destination block
(128 nodes) on the host; per-edge source rows are fetched with dma_gather and
scatter-added into PSUM via one-hot matmuls.  Node features needed by remote
gathers (xwn = dinv*x@W1, hn = dinv*h) are shard-computed and AllGathered.
"""

import numpy as np

import concourse.bass as bass
import concourse.mybir as mybir
from concourse import bacc, tile
from concourse.bass_utils import run_bass_kernel_spmd

F32 = mybir.dt.float32
I16 = mybir.dt.int16
SLOPE = 0.01
NCORES = 8
PART = 128
import os as _os0
NCHG = int(_os0.environ.get("K_NCHG", "6"))


def _leaky_stt(nc, out, x):
    # leaky_relu(x) = max(0.01*x, x)
    nc.vector.scalar_tensor_tensor(
        out, x, SLOPE, x, op0=mybir.AluOpType.mult, op1=mybir.AluOpType.max
    )


def build_program(NPC, HID, CPB, K_ELL, GBLK, enable_asserts=False):
    import os
    SKIP_EDGES = os.environ.get("K_SKIP_EDGES") == "1"
    SKIP_CC = os.environ.get("K_SKIP_CC") == "1"
    """One SPMD program shared by all 8 cores.

    NPC: nodes per core; NB = NPC/128 dst blocks per core.
    CPB: chunks (of 128 edges) per dst block (uniform, host-padded).
    K_ELL: ELL width for the degree pass.
    GBLK: dst blocks covered by one dma_gather.
    """
    NB = NPC // PART
    NG = NB // GBLK
    CAP = CPB * PART          # padded edges per block
    NCH = NB * CPB            # chunks per core
    TIDX = NB * CAP           # padded edges per core
    FC_HID = 128
    OUT = 2

    nc = bacc.Bacc(
        "TRN2",
        target_bir_lowering=False,
        debug=False,
        num_devices=NCORES,
        enable_asserts=enable_asserts,
    )

    # ---- I/O ----
    xT_d = nc.dram_tensor("xT", [PART, NPC], F32, kind="ExternalInput")
    W1_d = nc.dram_tensor("W1", [PART, HID], F32, kind="ExternalInput")
    b1t_d = nc.dram_tensor("b1t", [PART, HID], F32, kind="ExternalInput")
    W2t_d = nc.dram_tensor("W2t", [PART, HID], F32, kind="ExternalInput")
    b2t_d = nc.dram_tensor("b2t", [PART, 1], F32, kind="ExternalInput")
    fc1W_d = nc.dram_tensor("fc1W", [NPC, FC_HID], F32, kind="ExternalInput")
    fc1b_d = nc.dram_tensor("fc1b", [FC_HID, 1], F32, kind="ExternalInput")
    fc2W_d = nc.dram_tensor("fc2W", [FC_HID, OUT], F32, kind="ExternalInput")
    fc2b_d = nc.dram_tensor("fc2b", [1, OUT], F32, kind="ExternalInput")
    iota_d = nc.dram_tensor("iota", [PART, PART], F32, kind="ExternalInput")
    idx_d = nc.dram_tensor("idx", [PART, TIDX // 16], I16, kind="ExternalInput")
    wch_d = nc.dram_tensor("wch", [PART, NCH], F32, kind="ExternalInput")
    dlch_d = nc.dram_tensor("dlch", [PART, NCH], F32, kind="ExternalInput")
    well_d = nc.dram_tensor("well", [PART, NB * K_ELL], F32, kind="ExternalInput")
    out_d = nc.dram_tensor("out", [1, OUT], F32, kind="ExternalOutput")

    # ---- internal DRAM (collective bounce buffers) ----
    xwn_sh = nc.dram_tensor("xwn_sh", [PART, NB * HID], F32, kind="Internal")
    xwn_full = nc.dram_tensor(
        "xwn_full", [NCORES * PART, NB * HID], F32, kind="Internal",
        addr_space="Shared",
    )
    hn_sh = nc.dram_tensor("hn_sh", [PART, NB * HID], F32, kind="Internal")
    hn_full = nc.dram_tensor(
        "hn_full", [NCORES * PART, NB * HID], F32, kind="Internal",
        addr_space="Shared",
    )

    rg = [list(range(NCORES))]

    with tile.TileContext(nc) as tc:
        with (
            tc.tile_pool(name="const", bufs=1) as cpool,
            tc.tile_pool(name="big", bufs=1) as bigpool,
            tc.tile_pool(name="gat", bufs=8) as gatpool,
            tc.tile_pool(name="mask", bufs=6) as maskpool,
            tc.tile_pool(name="fin", bufs=4) as finpool,
            tc.tile_pool(name="psA", bufs=4, space="PSUM") as psA,
            tc.tile_pool(name="psB", bufs=2, space="PSUM") as psB,
        ):
            sync = nc.sync

            # ---- constant / input loads ----
            iota_sb = cpool.tile([PART, PART], F32, tag="iota")
            sync.dma_start(iota_sb[:], iota_d.ap())
            W1_sb = cpool.tile([PART, HID], F32, tag="W1")
            sync.dma_start(W1_sb[:], W1_d.ap())
            b1_sb = cpool.tile([PART, HID], F32, tag="b1")
            sync.dma_start(b1_sb[:], b1t_d.ap())
            W2t_sb = cpool.tile([PART, HID], F32, tag="W2t")
            sync.dma_start(W2t_sb[:], W2t_d.ap())
            b2_sb = cpool.tile([PART, 1], F32, tag="b2")
            sync.dma_start(b2_sb[:], b2t_d.ap())
            fc1b_sb = cpool.tile([FC_HID, 1], F32, tag="fc1b")
            sync.dma_start(fc1b_sb[:], fc1b_d.ap())
            fc2W_sb = cpool.tile([FC_HID, OUT], F32, tag="fc2W")
            sync.dma_start(fc2W_sb[:], fc2W_d.ap())
            fc2b_sb = cpool.tile([1, OUT], F32, tag="fc2b")
            sync.dma_start(fc2b_sb[:], fc2b_d.ap())

            xT_sb = bigpool.tile([PART, NPC], F32, tag="xT")
            sync.dma_start(xT_sb[:], xT_d.ap())
            well_sb = bigpool.tile([PART, NB * K_ELL], F32, tag="well")
            sync.dma_start(well_sb[:], well_d.ap())
            idx_sb = bigpool.tile([PART, TIDX // 16], I16, tag="idx")
            sync.dma_start(idx_sb[:], idx_d.ap())
            wch_sb = bigpool.tile([PART, NCH], F32, tag="wch")
            sync.dma_start(wch_sb[:], wch_d.ap())
            dlch_sb = bigpool.tile([PART, NCH], F32, tag="dlch")
            sync.dma_start(dlch_sb[:], dlch_d.ap())

            xwn_sb = bigpool.tile([PART, NB * HID], F32, tag="xwn")
            hn_sb = bigpool.tile([PART, NB * HID], F32, tag="hn")
            h2_sb = cpool.tile([PART, NB], F32, tag="h2")
            deg_sb = cpool.tile([PART, NB], F32, tag="deg")
            rec_sb = cpool.tile([PART, NB], F32, tag="rec")
            dinv_sb = cpool.tile([PART, NB], F32, tag="dinv")

            # ---- degree / dinv ----
            for b in range(NB):
                nc.vector.tensor_reduce(
                    out=deg_sb[:, b : b + 1],
                    in_=well_sb[:, b * K_ELL : (b + 1) * K_ELL],
                    axis=mybir.AxisListType.X,
                    op=mybir.AluOpType.add,
                )
            nc.vector.tensor_scalar_add(deg_sb[:], deg_sb[:], 1.0)
            nc.vector.reciprocal(rec_sb[:], deg_sb[:])
            nc.scalar.sqrt(dinv_sb[:], rec_sb[:])

            # ---- xwn = dinv * (x @ W1), own shard ----
            for nt in range(NB):
                ps = psB.tile([PART, HID], F32, tag="psb")
                nc.tensor.matmul(
                    ps[:],
                    xT_sb[:, nt * PART : (nt + 1) * PART],
                    W1_sb[:],
                    start=True,
                    stop=True,
                )
                nc.vector.tensor_scalar_mul(
                    xwn_sb[:, nt * HID : (nt + 1) * HID],
                    ps[:],
                    dinv_sb[:, nt : nt + 1],
                )
            sync.dma_start(xwn_sh.ap(), xwn_sb[:])
            if SKIP_CC:
                sync.dma_start(xwn_full.ap()[0:PART, :], xwn_sb[:])
            else:
                nc.gpsimd.collective_compute(
                    "AllGather",
                    mybir.AluOpType.bypass,
                    ins=[xwn_sh.ap().opt()],
                    outs=[xwn_full.ap().opt()],
                    replica_groups=rg,
                )

            def edge_pass(full_rows_ap, finalize):
                if SKIP_EDGES:
                    for B in range(NB):
                        ps = psA.tile([PART, HID], F32, tag="psagg")
                        nc.tensor.matmul(ps[:], iota_sb[:, 0:PART], W1_sb[:, :],
                                         start=True, stop=True)
                        finalize(B, ps)
                    return
                # NCHG chunks (128 edges each) per dma_gather; GBLK==NCHG here
                NCHG = GBLK
                assert CPB % NCHG == 0
                gt = None
                for B in range(NB):
                    ps = psA.tile([PART, HID], F32, tag="psagg")
                    for c in range(CPB):
                        gc = B * CPB + c
                        if gc % NCHG == 0:
                            u = gc // NCHG
                            nidx = NCHG * PART
                            gt = gatpool.tile([PART, NCHG * HID], F32, tag="gbuf")
                            nc.gpsimd.dma_gather(
                                gt[:].rearrange("p (c e) -> p c e", e=HID),
                                full_rows_ap,
                                idx_sb[:, u * (nidx // 16) : (u + 1) * (nidx // 16)],
                                nidx,
                                nidx,
                                HID,
                            )
                        o = gc % NCHG
                        m = maskpool.tile([PART, PART], F32, tag="mask")
                        nc.vector.tensor_scalar(
                            m[:],
                            iota_sb[:],
                            dlch_sb[:, gc : gc + 1],
                            wch_sb[:, gc : gc + 1],
                            mybir.AluOpType.is_equal,
                            mybir.AluOpType.mult,
                        )
                        nc.tensor.matmul(
                            ps[:],
                            m[:],
                            gt[:, o * HID : (o + 1) * HID],
                            start=(c == 0),
                            stop=(c == CPB - 1),
                        )
                    finalize(B, ps)

            # ---- layer 1 ----
            def fin1(B, ps):
                sl = slice(B * HID, (B + 1) * HID)
                t1 = finpool.tile([PART, HID], F32, tag="f1a")
                nc.vector.tensor_add(t1[:], ps[:], xwn_sb[:, sl])
                t2 = finpool.tile([PART, HID], F32, tag="f1b")
                nc.scalar.mul(t2[:], t1[:], dinv_sb[:, B : B + 1])
                t3 = finpool.tile([PART, HID], F32, tag="f1c")
                nc.vector.tensor_add(t3[:], t2[:], b1_sb[:])
                t4 = finpool.tile([PART, HID], F32, tag="f1d")
                _leaky_stt(nc, t4[:], t3[:])
                nc.vector.tensor_scalar_mul(
                    hn_sb[:, sl], t4[:], dinv_sb[:, B : B + 1]
                )

            edge_pass(
                xwn_full.ap().rearrange("a (b e) -> (a b) e", e=HID), fin1
            )

            sync.dma_start(hn_sh.ap(), hn_sb[:])
            if SKIP_CC:
                sync.dma_start(hn_full.ap()[0:PART, :], hn_sb[:])
            else:
                nc.gpsimd.collective_compute(
                    "AllGather",
                    mybir.AluOpType.bypass,
                    ins=[hn_sh.ap().opt()],
                    outs=[hn_full.ap().opt()],
                    replica_groups=rg,
                )

            # ---- layer 2 ----
            def fin2(B, ps):
                sl = slice(B * HID, (B + 1) * HID)
                t1 = finpool.tile([PART, HID], F32, tag="f2a")
                nc.vector.tensor_add(t1[:], ps[:], hn_sb[:, sl])
                t2 = finpool.tile([PART, HID], F32, tag="f2b")
                nc.scalar.mul(t2[:], t1[:], dinv_sb[:, B : B + 1])
                t3 = finpool.tile([PART, HID], F32, tag="f2c")
                nc.vector.tensor_mul(t3[:], t2[:], W2t_sb[:])
                z = finpool.tile([PART, 1], F32, tag="f2z")
                nc.vector.tensor_reduce(
                    out=z[:],
                    in_=t3[:],
                    axis=mybir.AxisListType.X,
                    op=mybir.AluOpType.add,
                )
                zb = finpool.tile([PART, 1], F32, tag="f2zb")
                nc.vector.tensor_add(zb[:], z[:], b2_sb[:])
                _leaky_stt(nc, h2_sb[:, B : B + 1], zb[:])

            edge_pass(
                hn_full.ap().rearrange("a (b e) -> (a b) e", e=HID), fin2
            )

            # ---- MLP head (this core's graph) ----
            fc1_sb = bigpool.tile([PART, NB * FC_HID], F32, tag="fc1")
            sync.dma_start(
                fc1_sb[:].rearrange("p (nt j) -> p nt j", j=FC_HID),
                fc1W_d.ap().rearrange("(nt p) j -> p nt j", p=PART),
            )
            ps1 = psB.tile([FC_HID, 1], F32, tag="psb")
            for nt in range(NB):
                nc.tensor.matmul(
                    ps1[:],
                    fc1_sb[:, nt * FC_HID : (nt + 1) * FC_HID],
                    h2_sb[:, nt : nt + 1],
                    start=(nt == 0),
                    stop=(nt == NB - 1),
                )
            rz = cpool.tile([FC_HID, 1], F32, tag="rz")
            nc.vector.tensor_add(rz[:], ps1[:], fc1b_sb[:])
            r1 = cpool.tile([FC_HID, 1], F32, tag="r1")
            _leaky_stt(nc, r1[:], rz[:])

            ps2 = psB.tile([1, OUT], F32, tag="psb")
            nc.tensor.matmul(ps2[:], r1[:], fc2W_sb[:], start=True, stop=True)
            oz = cpool.tile([1, OUT], F32, tag="oz")
            nc.vector.tensor_add(oz[:], ps2[:], fc2b_sb[:])
            mx = cpool.tile([1, 1], F32, tag="mx")
            nc.vector.tensor_reduce(
                out=mx[:], in_=oz[:], axis=mybir.AxisListType.X,
                op=mybir.AluOpType.max,
            )
            sh = cpool.tile([1, OUT], F32, tag="sh")
            nc.vector.tensor_scalar(
                sh[:], oz[:], mx[:], None, mybir.AluOpType.subtract
            )
            ex = cpool.tile([1, OUT], F32, tag="ex")
            nc.scalar.activation(ex[:], sh[:], mybir.ActivationFunctionType.Exp)
            sm = cpool.tile([1, 1], F32, tag="sm")
            nc.vector.tensor_reduce(
                out=sm[:], in_=ex[:], axis=mybir.AxisListType.X,
                op=mybir.AluOpType.add,
            )
            rc = cpool.tile([1, 1], F32, tag="rc")
            nc.vector.reciprocal(rc[:], sm[:])
            so = cpool.tile([1, OUT], F32, tag="so")
            nc.vector.tensor_scalar_mul(so[:], ex[:], rc[:])
            sync.dma_start(out_d.ap(), so[:])

    nc.compile()
    return nc


def host_prep(x, edge_index, edge_attr, W1, b1, W2, b2, fc1_W, fc1_b, fc2_W,
              fc2_b, NPC, HID):
    """Index-only host prep: bucket/pad edges by dst block, build gather
    indices, chunk metadata, and ELL weight layout.  No FP arithmetic on
    input values — only permutation / replication."""
    N = NCORES * NPC
    NB = NPC // PART
    src = np.asarray(edge_index[0]).astype(np.int64)
    dst = np.asarray(edge_index[1]).astype(np.int64)
    w = np.asarray(edge_attr)[:, 0].astype(np.float32)
    E = src.shape[0]

    gblk_id = dst >> 7                       # global 128-node block
    order = np.argsort(gblk_id, kind="stable")
    blk_counts = np.bincount(gblk_id, minlength=N // PART)
    CPB = max(1, int(-(-blk_counts.max() // PART)))
    CPB = -(-CPB // NCHG) * NCHG
    CAP = CPB * PART

    src_pad = np.zeros((NCORES, NB * CAP), np.int64)
    w_pad = np.zeros((NCORES, NB * CAP), np.float32)
    dl_pad = np.zeros((NCORES, NB * CAP), np.float32)
    starts = np.zeros(N // PART + 1, np.int64)
    starts[1:] = np.cumsum(blk_counts)
    for gb in range(N // PART):
        k, b = divmod(gb, NB)
        seg = order[starts[gb] : starts[gb + 1]]
        cnt = seg.shape[0]
        off = b * CAP
        src_pad[k, off : off + cnt] = src[seg]
        w_pad[k, off : off + cnt] = w[seg]
        dl_pad[k, off : off + cnt] = (dst[seg] & 127).astype(np.float32)

    # gather-row mapping for the blocked [core*128+p, nt*HID] DRAM layout
    def grow(n):
        k2 = n // NPC
        r = n % NPC
        return k2 * NPC + (r % PART) * NB + (r // PART)

    idx16 = grow(src_pad).astype(np.int16)          # [NCORES, NB*CAP]
    T = NB * CAP
    idx_sb = np.tile(
        idx16.reshape(NCORES, T // 16, 16).transpose(0, 2, 1), (1, 8, 1)
    )                                               # [NCORES, 128, T//16]
    NCH = NB * CPB
    wch = w_pad.reshape(NCORES, NCH, PART).transpose(0, 2, 1).copy()
    dlch = dl_pad.reshape(NCORES, NCH, PART).transpose(0, 2, 1).copy()

    # ELL layout of w per destination node (for the degree pass)
    deg_cnt = np.bincount(dst, minlength=N)
    K_ELL = max(1, int(deg_cnt.max()))
    order2 = np.argsort(dst, kind="stable")
    nstarts = np.zeros(N + 1, np.int64)
    nstarts[1:] = np.cumsum(deg_cnt)
    ranks = np.arange(E, dtype=np.int64) - nstarts[dst[order2]]
    well_flat = np.zeros((N, K_ELL), np.float32)
    well_flat[dst[order2], ranks] = w[order2]
    well = (
        well_flat.reshape(NCORES, NB, PART, K_ELL)
        .transpose(0, 2, 1, 3)
        .reshape(NCORES, PART, NB * K_ELL)
        .copy()
    )

    x = np.asarray(x, dtype=np.float32)
    W1a = np.asarray(W1, dtype=np.float32)
    HIDp = W1a.shape[1]
    assert HIDp == HID
    b1t = np.tile(np.asarray(b1, np.float32).reshape(1, HID), (PART, 1))
    W2t = np.tile(np.asarray(W2, np.float32).reshape(1, HID), (PART, 1))
    b2t = np.full((PART, 1), np.asarray(b2, np.float32).reshape(-1)[0], np.float32)
    fc1W = np.ascontiguousarray(np.asarray(fc1_W, np.float32))
    fc1b = np.asarray(fc1_b, np.float32).reshape(-1, 1)
    fc2W = np.ascontiguousarray(np.asarray(fc2_W, np.float32))
    fc2b = np.asarray(fc2_b, np.float32).reshape(1, -1)
    iota = np.tile(np.arange(PART, dtype=np.float32), (PART, 1))

    in_maps = []
    for k in range(NCORES):
        xT = np.ascontiguousarray(x[k * NPC : (k + 1) * NPC].T)
        in_maps.append(
            {
                "xT": xT,
                "W1": W1a,
                "b1t": b1t,
                "W2t": W2t,
                "b2t": b2t,
                "fc1W": fc1W,
                "fc1b": fc1b,
                "fc2W": fc2W,
                "fc2b": fc2b,
                "iota": iota,
                "idx": np.ascontiguousarray(idx_sb[k]),
                "wch": np.ascontiguousarray(wch[k]),
                "dlch": np.ascontiguousarray(dlch[k]),
                "well": np.ascontiguousarray(well[k]),
            }
        )
    return in_maps, CPB, K_ELL


_CACHE = {}


def _get_program(NPC, HID, CPB, K_ELL, GBLK):
    import os
    key = (NPC, HID, CPB, K_ELL, GBLK,
           os.environ.get("K_SKIP_EDGES"), os.environ.get("K_SKIP_CC"))
    if key not in _CACHE:
        _CACHE[key] = build_program(NPC, HID, CPB, K_ELL, GBLK)
    return _CACHE[key]


def kernel(x, edge_index, edge_attr, W1, b1, W2, b2, fc1_W, fc1_b, fc2_W,
           fc2_b, num_graphs, _trace=False, _tmpdir=None):
    N = x.shape[0]
    NPC = N // NCORES
    HID = np.asarray(W1).shape[1]
    import os as _os
    in_maps, CPB, K_ELL = host_prep(
        x, edge_index, edge_attr, W1, b1, W2, b2, fc1_W, fc1_b, fc2_W, fc2_b,
        NPC, HID,
    )
    NB = NPC // PART
    GBLK = NCHG
    nc = _get_program(NPC, HID, CPB, K_ELL, GBLK)
    res = run_bass_kernel_spmd(
        nc, in_maps, core_ids=list(range(NCORES)), trace=_trace, tmpdir=_tmpdir
    )
    out = np.concatenate([res.results[k]["out"] for k in range(NCORES)], axis=0)
    if _trace:
        kernel._last_exec_time_ns = res.exec_time_ns
        kernel._last_results = res
    return out.astype(np.float32)


# revision 10
# speedup vs baseline: 1.0052x; 1.0052x over previous
"""Trainium2 Bass kernel: 2-layer GCN (PyG GCNConv w/ edge weights) + MLP head.

Sharding: core k owns destination nodes [k*NPC, (k+1)*NPC) — which is exactly
graph k (NODES_PER_G == N/8).  All edges are bucketed by destination block
(128 nodes) on the host; per-edge source rows are fetched with dma_gather and
scatter-added into PSUM via one-hot matmuls.  Node features needed by remote
gathers (xwn = dinv*x@W1, hn = dinv*h) are shard-computed and AllGathered.
"""

import numpy as np

import concourse.bass as bass
import concourse.mybir as mybir
from concourse import bacc, tile
from concourse.bass_utils import run_bass_kernel_spmd

F32 = mybir.dt.float32
I16 = mybir.dt.int16
SLOPE = 0.01
NCORES = 8
PART = 128
import os as _os0
NCHG = int(_os0.environ.get("K_NCHG", "6"))


def _leaky_stt(nc, out, x):
    # leaky_relu(x) = max(0.01*x, x)
    nc.vector.scalar_tensor_tensor(
        out, x, SLOPE, x, op0=mybir.AluOpType.mult, op1=mybir.AluOpType.max
    )


def build_program(NPC, HID, CPB, K_ELL, GBLK, enable_asserts=False):
    import os
    SKIP_EDGES = os.environ.get("K_SKIP_EDGES") == "1"
    SKIP_CC = os.environ.get("K_SKIP_CC") == "1"
    """One SPMD program shared by all 8 cores.

    NPC: nodes per core; NB = NPC/128 dst blocks per core.
    CPB: chunks (of 128 edges) per dst block (uniform, host-padded).
    K_ELL: ELL width for the degree pass.
    GBLK: dst blocks covered by one dma_gather.
    """
    NB = NPC // PART
    NG = NB // GBLK
    CAP = CPB * PART          # padded edges per block
    NCH = NB * CPB            # chunks per core
    TIDX = NB * CAP           # padded edges per core
    FC_HID = 128
    OUT = 2

    nc = bacc.Bacc(
        "TRN2",
        target_bir_lowering=False,
        debug=False,
        num_devices=NCORES,
        enable_asserts=enable_asserts,
    )

    # ---- I/O ----
    xT_d = nc.dram_tensor("xT", [PART, NPC], F32, kind="ExternalInput")
    W1_d = nc.dram_tensor("W1", [PART, HID], F32, kind="ExternalInput")
    b1t_d = nc.dram_tensor("b1t", [PART, HID], F32, kind="ExternalInput")
    W2t_d = nc.dram_tensor("W2t", [PART, HID], F32, kind="ExternalInput")
    b2t_d = nc.dram_tensor("b2t", [PART, 1], F32, kind="ExternalInput")
    fc1W_d = nc.dram_tensor("fc1W", [NPC, FC_HID], F32, kind="ExternalInput")
    fc1b_d = nc.dram_tensor("fc1b", [FC_HID, 1], F32, kind="ExternalInput")
    fc2W_d = nc.dram_tensor("fc2W", [FC_HID, OUT], F32, kind="ExternalInput")
    fc2b_d = nc.dram_tensor("fc2b", [1, OUT], F32, kind="ExternalInput")
    iota_d = nc.dram_tensor("iota", [PART, PART], F32, kind="ExternalInput")
    idx_d = nc.dram_tensor("idx", [PART, TIDX // 16], I16, kind="ExternalInput")
    wch_d = nc.dram_tensor("wch", [PART, NCH], F32, kind="ExternalInput")
    dlch_d = nc.dram_tensor("dlch", [PART, NCH], F32, kind="ExternalInput")
    well_d = nc.dram_tensor("well", [PART, NB * K_ELL], F32, kind="ExternalInput")
    out_d = nc.dram_tensor("out", [1, OUT], F32, kind="ExternalOutput")

    # ---- internal DRAM (collective bounce buffers) ----
    xwn_sh = nc.dram_tensor("xwn_sh", [PART, NB * HID], F32, kind="Internal")
    xwn_full = nc.dram_tensor(
        "xwn_full", [NCORES * PART, NB * HID], F32, kind="Internal",
        addr_space="Shared",
    )
    hn_sh = nc.dram_tensor("hn_sh", [PART, NB * HID], F32, kind="Internal")
    hn_full = nc.dram_tensor(
        "hn_full", [NCORES * PART, NB * HID], F32, kind="Internal",
        addr_space="Shared",
    )

    rg = [list(range(NCORES))]

    with tile.TileContext(nc) as tc:
        with (
            tc.tile_pool(name="const", bufs=1) as cpool,
            tc.tile_pool(name="big", bufs=1) as bigpool,
            tc.tile_pool(name="gat", bufs=8) as gatpool,
            tc.tile_pool(name="mask", bufs=6) as maskpool,
            tc.tile_pool(name="fin", bufs=4) as finpool,
            tc.tile_pool(name="psA", bufs=4, space="PSUM") as psA,
            tc.tile_pool(name="psB", bufs=2, space="PSUM") as psB,
        ):
            sync = nc.sync

            # ---- constant / input loads ----
            iota_sb = cpool.tile([PART, PART], F32, tag="iota")
            sync.dma_start(iota_sb[:], iota_d.ap())
            W1_sb = cpool.tile([PART, HID], F32, tag="W1")
            sync.dma_start(W1_sb[:], W1_d.ap())
            b1_sb = cpool.tile([PART, HID], F32, tag="b1")
            sync.dma_start(b1_sb[:], b1t_d.ap())
            W2t_sb = cpool.tile([PART, HID], F32, tag="W2t")
            sync.dma_start(W2t_sb[:], W2t_d.ap())
            b2_sb = cpool.tile([PART, 1], F32, tag="b2")
            sync.dma_start(b2_sb[:], b2t_d.ap())
            fc1b_sb = cpool.tile([FC_HID, 1], F32, tag="fc1b")
            sync.dma_start(fc1b_sb[:], fc1b_d.ap())
            fc2W_sb = cpool.tile([FC_HID, OUT], F32, tag="fc2W")
            sync.dma_start(fc2W_sb[:], fc2W_d.ap())
            fc2b_sb = cpool.tile([1, OUT], F32, tag="fc2b")
            sync.dma_start(fc2b_sb[:], fc2b_d.ap())

            xT_sb = bigpool.tile([PART, NPC], F32, tag="xT")
            sync.dma_start(xT_sb[:], xT_d.ap())
            well_sb = bigpool.tile([PART, NB * K_ELL], F32, tag="well")
            sync.dma_start(well_sb[:], well_d.ap())
            idx_sb = bigpool.tile([PART, TIDX // 16], I16, tag="idx")
            sync.dma_start(idx_sb[:], idx_d.ap())
            wch_sb = bigpool.tile([PART, NCH], F32, tag="wch")
            sync.dma_start(wch_sb[:], wch_d.ap())
            dlch_sb = bigpool.tile([PART, NCH], F32, tag="dlch")
            sync.dma_start(dlch_sb[:], dlch_d.ap())

            xwn_sb = bigpool.tile([PART, NB * HID], F32, tag="xwn")
            hn_sb = bigpool.tile([PART, NB * HID], F32, tag="hn")
            h2_sb = cpool.tile([PART, NB], F32, tag="h2")
            deg_sb = cpool.tile([PART, NB], F32, tag="deg")
            rec_sb = cpool.tile([PART, NB], F32, tag="rec")
            dinv_sb = cpool.tile([PART, NB], F32, tag="dinv")

            # ---- degree / dinv ----
            for b in range(NB):
                nc.vector.tensor_reduce(
                    out=deg_sb[:, b : b + 1],
                    in_=well_sb[:, b * K_ELL : (b + 1) * K_ELL],
                    axis=mybir.AxisListType.X,
                    op=mybir.AluOpType.add,
                )
            nc.vector.tensor_scalar_add(deg_sb[:], deg_sb[:], 1.0)
            nc.vector.reciprocal(rec_sb[:], deg_sb[:])
            nc.scalar.sqrt(dinv_sb[:], rec_sb[:])

            # ---- xwn = dinv * (x @ W1), own shard ----
            for nt in range(NB):
                ps = psB.tile([PART, HID], F32, tag="psb")
                nc.tensor.matmul(
                    ps[:],
                    xT_sb[:, nt * PART : (nt + 1) * PART],
                    W1_sb[:],
                    start=True,
                    stop=True,
                )
                nc.vector.tensor_scalar_mul(
                    xwn_sb[:, nt * HID : (nt + 1) * HID],
                    ps[:],
                    dinv_sb[:, nt : nt + 1],
                )
            sync.dma_start(xwn_sh.ap(), xwn_sb[:])
            if SKIP_CC:
                sync.dma_start(xwn_full.ap()[0:PART, :], xwn_sb[:])
            else:
                nc.gpsimd.collective_compute(
                    "AllGather",
                    mybir.AluOpType.bypass,
                    ins=[xwn_sh.ap().opt()],
                    outs=[xwn_full.ap().opt()],
                    replica_groups=rg,
                )

            def edge_pass(full_rows_ap, finalize):
                if SKIP_EDGES:
                    for B in range(NB):
                        ps = psA.tile([PART, HID], F32, tag="psagg")
                        nc.tensor.matmul(ps[:], iota_sb[:, 0:PART], W1_sb[:, :],
                                         start=True, stop=True)
                        finalize(B, ps)
                    return
                # NCHG chunks (128 edges each) per dma_gather; GBLK==NCHG here
                NCHG = GBLK
                assert CPB % NCHG == 0
                gt = None
                for B in range(NB):
                    ps = psA.tile([PART, HID], F32, tag="psagg")
                    for c in range(CPB):
                        gc = B * CPB + c
                        if gc % NCHG == 0:
                            u = gc // NCHG
                            nidx = NCHG * PART
                            gt = gatpool.tile([PART, NCHG * HID], F32, tag="gbuf")
                            nc.gpsimd.dma_gather(
                                gt[:].rearrange("p (c e) -> p c e", e=HID),
                                full_rows_ap,
                                idx_sb[:, u * (nidx // 16) : (u + 1) * (nidx // 16)],
                                nidx,
                                nidx,
                                HID,
                            )
                        o = gc % NCHG
                        m = maskpool.tile([PART, PART], F32, tag="mask")
                        nc.vector.tensor_scalar(
                            m[:],
                            iota_sb[:],
                            dlch_sb[:, gc : gc + 1],
                            wch_sb[:, gc : gc + 1],
                            mybir.AluOpType.is_equal,
                            mybir.AluOpType.mult,
                        )
                        nc.tensor.matmul(
                            ps[:],
                            m[:],
                            gt[:, o * HID : (o + 1) * HID],
                            start=(c == 0),
                            stop=(c == CPB - 1),
                        )
                    finalize(B, ps)

            # ---- layer 1 ----
            def fin1(B, ps):
                sl = slice(B * HID, (B + 1) * HID)
                t1 = finpool.tile([PART, HID], F32, tag="f1a")
                nc.vector.tensor_add(t1[:], ps[:], xwn_sb[:, sl])
                t2 = finpool.tile([PART, HID], F32, tag="f1b")
                nc.scalar.mul(t2[:], t1[:], dinv_sb[:, B : B + 1])
                t3 = finpool.tile([PART, HID], F32, tag="f1c")
                nc.vector.tensor_add(t3[:], t2[:], b1_sb[:])
                t4 = finpool.tile([PART, HID], F32, tag="f1d")
                _leaky_stt(nc, t4[:], t3[:])
                nc.vector.tensor_scalar_mul(
                    hn_sb[:, sl], t4[:], dinv_sb[:, B : B + 1]
                )

            edge_pass(
                xwn_full.ap().rearrange("a (b e) -> (a b) e", e=HID), fin1
            )

            sync.dma_start(hn_sh.ap(), hn_sb[:])
            if SKIP_CC:
                sync.dma_start(hn_full.ap()[0:PART, :], hn_sb[:])
            else:
                nc.gpsimd.collective_compute(
                    "AllGather",
                    mybir.AluOpType.bypass,
                    ins=[hn_sh.ap().opt()],
                    outs=[hn_full.ap().opt()],
                    replica_groups=rg,
                )

            # ---- layer 2 ----
            def fin2(B, ps):
                sl = slice(B * HID, (B + 1) * HID)
                t1 = finpool.tile([PART, HID], F32, tag="f2a")
                nc.vector.tensor_add(t1[:], ps[:], hn_sb[:, sl])
                t2 = finpool.tile([PART, HID], F32, tag="f2b")
                nc.scalar.mul(t2[:], t1[:], dinv_sb[:, B : B + 1])
                t3 = finpool.tile([PART, HID], F32, tag="f2c")
                nc.vector.tensor_mul(t3[:], t2[:], W2t_sb[:])
                z = finpool.tile([PART, 1], F32, tag="f2z")
                nc.vector.tensor_reduce(
                    out=z[:],
                    in_=t3[:],
                    axis=mybir.AxisListType.X,
                    op=mybir.AluOpType.add,
                )
                zb = finpool.tile([PART, 1], F32, tag="f2zb")
                nc.vector.tensor_add(zb[:], z[:], b2_sb[:])
                _leaky_stt(nc, h2_sb[:, B : B + 1], zb[:])

            edge_pass(
                hn_full.ap().rearrange("a (b e) -> (a b) e", e=HID), fin2
            )

            # ---- MLP head (this core's graph) ----
            fc1_sb = bigpool.tile([PART, NB * FC_HID], F32, tag="fc1")
            sync.dma_start(
                fc1_sb[:].rearrange("p (nt j) -> p nt j", j=FC_HID),
                fc1W_d.ap().rearrange("(nt p) j -> p nt j", p=PART),
            )
            ps1 = psB.tile([FC_HID, 1], F32, tag="psb")
            for nt in range(NB):
                nc.tensor.matmul(
                    ps1[:],
                    fc1_sb[:, nt * FC_HID : (nt + 1) * FC_HID],
                    h2_sb[:, nt : nt + 1],
                    start=(nt == 0),
                    stop=(nt == NB - 1),
                )
            rz = cpool.tile([FC_HID, 1], F32, tag="rz")
            nc.vector.tensor_add(rz[:], ps1[:], fc1b_sb[:])
            r1 = cpool.tile([FC_HID, 1], F32, tag="r1")
            _leaky_stt(nc, r1[:], rz[:])

            ps2 = psB.tile([1, OUT], F32, tag="psb")
            nc.tensor.matmul(ps2[:], r1[:], fc2W_sb[:], start=True, stop=True)
            oz = cpool.tile([1, OUT], F32, tag="oz")
            nc.vector.tensor_add(oz[:], ps2[:], fc2b_sb[:])
            mx = cpool.tile([1, 1], F32, tag="mx")
            nc.vector.tensor_reduce(
                out=mx[:], in_=oz[:], axis=mybir.AxisListType.X,
                op=mybir.AluOpType.max,
            )
            sh = cpool.tile([1, OUT], F32, tag="sh")
            nc.vector.tensor_scalar(
                sh[:], oz[:], mx[:], None, mybir.AluOpType.subtract
            )
            ex = cpool.tile([1, OUT], F32, tag="ex")
            nc.scalar.activation(ex[:], sh[:], mybir.ActivationFunctionType.Exp)
            sm = cpool.tile([1, 1], F32, tag="sm")
            nc.vector.tensor_reduce(
                out=sm[:], in_=ex[:], axis=mybir.AxisListType.X,
                op=mybir.AluOpType.add,
            )
            rc = cpool.tile([1, 1], F32, tag="rc")
            nc.vector.reciprocal(rc[:], sm[:])
            so = cpool.tile([1, OUT], F32, tag="so")
            nc.vector.tensor_scalar_mul(so[:], ex[:], rc[:])
            sync.dma_start(out_d.ap(), so[:])

    nc.compile()
    return nc


def host_prep(x, edge_index, edge_attr, W1, b1, W2, b2, fc1_W, fc1_b, fc2_W,
              fc2_b, NPC, HID):
    """Index-only host prep: bucket/pad edges by dst block, build gather
    indices, chunk metadata, and ELL weight layout.  No FP arithmetic on
    input values — only permutation / replication."""
    N = NCORES * NPC
    NB = NPC // PART
    src = np.asarray(edge_index[0]).astype(np.int64)
    dst = np.asarray(edge_index[1]).astype(np.int64)
    w = np.asarray(edge_attr)[:, 0].astype(np.float32)
    E = src.shape[0]

    gblk_id = dst >> 7                       # global 128-node block
    order = np.argsort(gblk_id, kind="stable")
    blk_counts = np.bincount(gblk_id, minlength=N // PART)
    CPB = max(1, int(-(-blk_counts.max() // PART)))
    CPB = -(-CPB // NCHG) * NCHG
    CAP = CPB * PART

    src_pad = np.zeros((NCORES, NB * CAP), np.int64)
    w_pad = np.zeros((NCORES, NB * CAP), np.float32)
    dl_pad = np.zeros((NCORES, NB * CAP), np.float32)
    starts = np.zeros(N // PART + 1, np.int64)
    starts[1:] = np.cumsum(blk_counts)
    for gb in range(N // PART):
        k, b = divmod(gb, NB)
        seg = order[starts[gb] : starts[gb + 1]]
        cnt = seg.shape[0]
        off = b * CAP
        src_pad[k, off : off + cnt] = src[seg]
        w_pad[k, off : off + cnt] = w[seg]
        dl_pad[k, off : off + cnt] = (dst[seg] & 127).astype(np.float32)

    # gather-row mapping for the blocked [core*128+p, nt*HID] DRAM layout
    def grow(n):
        k2 = n // NPC
        r = n % NPC
        return k2 * NPC + (r % PART) * NB + (r // PART)

    idx16 = grow(src_pad).astype(np.int16)          # [NCORES, NB*CAP]
    T = NB * CAP
    idx_sb = np.tile(
        idx16.reshape(NCORES, T // 16, 16).transpose(0, 2, 1), (1, 8, 1)
    )                                               # [NCORES, 128, T//16]
    NCH = NB * CPB
    wch = w_pad.reshape(NCORES, NCH, PART).transpose(0, 2, 1).copy()
    dlch = dl_pad.reshape(NCORES, NCH, PART).transpose(0, 2, 1).copy()

    # ELL layout of w per destination node (for the degree pass)
    deg_cnt = np.bincount(dst, minlength=N)
    K_ELL = max(1, int(deg_cnt.max()))
    order2 = np.argsort(dst, kind="stable")
    nstarts = np.zeros(N + 1, np.int64)
    nstarts[1:] = np.cumsum(deg_cnt)
    ranks = np.arange(E, dtype=np.int64) - nstarts[dst[order2]]
    well_flat = np.zeros((N, K_ELL), np.float32)
    well_flat[dst[order2], ranks] = w[order2]
    well = (
        well_flat.reshape(NCORES, NB, PART, K_ELL)
        .transpose(0, 2, 1, 3)
        .reshape(NCORES, PART, NB * K_ELL)
        .copy()
    )

    x = np.asarray(x, dtype=np.float32)
    W1a = np.asarray(W1, dtype=np.float32)
    HIDp = W1a.shape[1]
    assert HIDp == HID
    b1t = np.tile(np.asarray(b1, np.float32).reshape(1, HID), (PART, 1))
    W2t = np.tile(np.asarray(W2, np.float32).reshape(1, HID), (PART, 1))
    b2t = np.full((PART, 1), np.asarray(b2, np.float32).reshape(-1)[0], np.float32)
    fc1W = np.ascontiguousarray(np.asarray(fc1_W, np.float32))
    fc1b = np.asarray(fc1_b, np.float32).reshape(-1, 1)
    fc2W = np.ascontiguousarray(np.asarray(fc2_W, np.float32))
    fc2b = np.asarray(fc2_b, np.float32).reshape(1, -1)
    iota = np.tile(np.arange(PART, dtype=np.float32), (PART, 1))

    in_maps = []
    for k in range(NCORES):
        xT = np.ascontiguousarray(x[k * NPC : (k + 1) * NPC].T)
        in_maps.append(
            {
                "xT": xT,
                "W1": W1a,
                "b1t": b1t,
                "W2t": W2t,
                "b2t": b2t,
                "fc1W": fc1W,
                "fc1b": fc1b,
                "fc2W": fc2W,
                "fc2b": fc2b,
                "iota": iota,
                "idx": np.ascontiguousarray(idx_sb[k]),
                "wch": np.ascontiguousarray(wch[k]),
                "dlch": np.ascontiguousarray(dlch[k]),
                "well": np.ascontiguousarray(well[k]),
            }
        )
    return in_maps, CPB, K_ELL


_CACHE = {}


def _get_program(NPC, HID, CPB, K_ELL, GBLK):
    import os
    key = (NPC, HID, CPB, K_ELL, GBLK,
           os.environ.get("K_SKIP_EDGES"), os.environ.get("K_SKIP_CC"))
    if key not in _CACHE:
        _CACHE[key] = build_program(NPC, HID, CPB, K_ELL, GBLK)
    return _CACHE[key]


def kernel(x, edge_index, edge_attr, W1, b1, W2, b2, fc1_W, fc1_b, fc2_W,
           fc2_b, num_graphs, _trace=False, _tmpdir=None):
    N = x.shape[0]
    NPC = N // NCORES
    HID = np.asarray(W1).shape[1]
    import os as _os
    in_maps, CPB, K_ELL = host_prep(
        x, edge_index, edge_attr, W1, b1, W2, b2, fc1_W, fc1_b, fc2_W, fc2_b,
        NPC, HID,
    )
    NB = NPC // PART
    GBLK = NCHG
    nc = _get_program(NPC, HID, CPB, K_ELL, GBLK)
    res = run_bass_kernel_spmd(
        nc, in_maps, core_ids=list(range(NCORES)), trace=_trace, tmpdir=_tmpdir
    )
    out = np.concatenate([res.results[k]["out"] for k in range(NCORES)], axis=0)
    if _trace:
        kernel._last_exec_time_ns = res.exec_time_ns
        kernel._last_results = res
    return out.astype(np.float32)
